# revision 20
# baseline (speedup 1.0000x reference)
"""Multi-head causal attention (B=2, T=2048, D=1024, H=16) on 8 Trainium2
NeuronCores.

Sharding: batch x head-group data/tensor parallel. Core c handles batch
c//4 and heads (c%4)*4 .. +4: W_qkv is split column-wise per head group,
W_o row-wise; each core computes attention for its local heads and a
partial output projection. The host sums the 4 partials per batch
(row-parallel W_o reduction) and stacks the two batches.

Per-core device kernel (fp16 data path, fp32 PSUM accumulate):
  - Inputs ride in as a handful of batched 3D-AP DMA waves (one
    dma_start per column range over all eight 128-row chunks of the
    [D, 2816] comb layout [Wq_p0|Wk_p0|Wq_p1|Wk_p1|Wv|xT]), spread
    across the SP/ACT/Pool DGE queues so descriptor issue (~600 ns
    each) never serializes the head.
  - Q/K live as head-PAIR tiles [128, T]: partitions 0:64 = even head
    dims, 64:128 = odd. The two QK matmuls of a pair run K=64
    row-tiled into ONE two-bank PSUM tile [128, 1024] (A cols 0:512,
    B cols 512:1024) so both banks free together and the matmuls
    execute concurrently in the PE array's row halves.
  - exp(s/8) on ACT: one fused [128, 1024] op per full k-tile (both
    heads), two ops for the column-restricted diagonal tiles; causal
    mask multiply on DVE for the diagonal 128-col block.
  - AV accumulates [65, 512] per head (V tiles carry a ones column so
    the softmax denominator falls out as row 64). Each av is copied to
    SBUF right away (ar tiles), releasing the PSUM bank, so the whole
    normalization chain runs off the critical path.
  - Normalization per chunk (4 heads packed at partitions 0/32/64/96):
    den rows gathered on GpSimd, 1/den = exp(-ln d) on ACT (one LN +
    one EXP per chunk), K=1 ones-matmul broadcast on PE into a shared
    [128, 512] bc tile (two heads per bank via out base partition),
    multiply ar * bc into the fp32r attnT on DVE.
  - W_o projection (fp32r) per 128-token step, PSUM -> SBUF fp16
    copies on DVE (tail: ACT), one batched [128, 1024] out-DMA per
    step.
  - Tail: per-proc drain waits + a single fan-out barrier + semaphore
    clear instead of the stock double butterfly barrier.
  All projection / V-tile / W_o / norm quanta ride as fillers at chosen
  slots inside the attention kt loops, sized so the PE queue stays
  dense while ACT works through the exp stream.
"""
import sys

for _p in ("/opt/trn_rl_repo", "/root/.axon_site/_ro/trn_rl_repo"):
    if _p not in sys.path:
        sys.path.insert(0, _p)

import numpy as np
import concourse.bass as bass
import concourse.mybir as mybir
import concourse.tile as tile
from concourse.vector_clock import ScopedClock
from concourse.bass_utils import run_bass_kernel_spmd

F32 = mybir.dt.float32
F32R = mybir.dt.float32r
F16 = mybir.dt.float16
AF = mybir.ActivationFunctionType

B, T, D = 2, 2048, 1024
N_CORES = 8
HPC = 4            # heads per core
HL = HPC * 64      # 256 local head dims
NKT = T // 128     # 16 k-tiles per head
NQC = T // 512     # 4 q-chunks
CW = 2816          # comb columns: Wq0|Wk0|Wq1|Wk1|Wv|xT


class FixedTileContext(tile.TileContext):
    """Works around this walrus build's 1-sync-wait-per-instruction limit.

    1. `_add_instruction`: peel extra waits off any instruction onto
       standalone single-wait nops emitted just before it on the same
       engine (the sequencer executes them in order).
    2. `_drain_and_barrier`: per-proc single-wait sync nops, then a
       single fan-out barrier (sync increments, every other engine takes
       one wait) instead of two full butterfly barriers, then the
       semaphore clear on gpsimd.
    """

    def _add_instruction(self, inst):
        si = inst.sync_info
        if si is not None:
            waits = list(si.on_wait)
            if len(waits) > 1:
                eng = getattr(inst, "engine", None)
                eng_obj = self.nc.engines.get(eng) if eng is not None else None
                if eng_obj is not None:
                    for w in waits[:-1]:
                        nop = eng_obj.nop()
                        nop.ins.sync_info = mybir.SyncInfo(on_wait=[w], on_update=[])
                    inst.sync_info = mybir.SyncInfo(
                        on_wait=[waits[-1]], on_update=list(si.on_update)
                    )
        super()._add_instruction(inst)

    def _drain_and_barrier(self, tick_clock, wait_clock):
        nc = self.nc
        vec = tick_clock.global_clock
        for proc in range(len(vec)):
            t = vec[proc]
            if t <= 0:
                continue
            partial = ScopedClock()
            partial.require_at_least(None, proc, t)
            w = nc.sync.nop()
            wait_clock.add_sem_waits(w.ins, partial)
        nc.sync.drain()
        # fan-out: sync (which waited on every proc above) releases the
        # other engines with one semaphore; each takes a single wait.
        bsem = nc.alloc_semaphore("tail_fanout")
        nc.sync.sem_inc(bsem)
        for eng in nc.engines.values():
            if eng is not nc.sync:
                eng.wait_ge(bsem, 1)
        assert self.sems is not None
        popped = nc._tile_sem_poison_stack.pop()
        assert popped is self._sem_poison
        sems = list(self.sems.allocated().values())
        nc.clear_and_free_semaphores(sems)
        nc.gpsimd.sem_clear(range(bsem.num, bsem.num + 1))


def build_nc():
    nc = bass.Bass()
    cx = nc.declare_dram_parameter("cx", [D, CW], F16, isOutput=False)
    wo = nc.declare_dram_parameter("wo", [HL, D], F32R, isOutput=False)
    consts = nc.declare_dram_parameter("consts", [128, 128], F16, isOutput=False)
    out = nc.declare_dram_parameter("out", [T, D], F16, isOutput=True)

    cxr = cx[:].rearrange("(k p) c -> p k c", p=128)      # [128, 8, 2816]

    with FixedTileContext(nc) as tc:
        with tc.tile_pool(name="persist", bufs=1) as pp, \
             tc.tile_pool(name="work", bufs=4) as wp, \
             tc.tile_pool(name="nwork", bufs=4) as nwp, \
             tc.tile_pool(name="psum", bufs=2, space="PSUM") as psp:
            comb = pp.tile([128, 8 * CW], F16, tag="comb")
            combv = comb[:].rearrange("p (k c) -> p k c", c=CW)
            consts_t = pp.tile([128, 128], F16, tag="consts")
            ones_t = pp.tile([128, 512], F16, tag="ones")
            wo_t = [pp.tile([128, D], F32R, tag=f"wo{c}", name=f"wo{c}")
                    for c in range(2)]

            # ---- input DMA waves ----
            # scalar + gpsimd have the shortest sequencer preambles (their
            # first descriptors hit the DGE ~2 us in, vs ~7 us for sync),
            # so the first-projection-critical columns ride there; sync
            # carries the late-needed bulk.
            nc.scalar.dma_start(combv[:, :, 0:256], cxr[:, :, 0:256])
            nc.scalar.dma_start(combv[:, 0:4, 768:1280], cxr[:, 0:4, 768:1280])
            nc.gpsimd.memset(ones_t[:], 1.0)     # warmup dep, ~0.5 us
            nc.gpsimd.dma_start(combv[:, 4:8, 768:1280], cxr[:, 4:8, 768:1280])
            nc.gpsimd.dma_start(combv[:, :, 256:768], cxr[:, :, 256:768])
            nc.sync.dma_start(consts_t[:], consts[:])
            nc.sync.dma_start(combv[:, :, 1280:1792], cxr[:, :, 1280:1792])
            for c in range(2):
                nc.sync.dma_start(wo_t[c][:], wo[c * 128:(c + 1) * 128, :])
            nc.sync.dma_start(combv[:, :, 1792:2304], cxr[:, :, 1792:2304])
            nc.sync.dma_start(combv[:, :, 2304:2816], cxr[:, :, 2304:2816])

            # HAM warm-up: dummy N=512 accumulation chain on the ones tile
            # keeps the PE busy while the first input waves land so the
            # clock gate opens (cold K=4/8 halves the PE clock).
            wu = psp.tile([128, 512], F32, tag="misc", name="wu", bufs=2)
            for i in range(8):
                nc.tensor.matmul(
                    wu[:], ones_t[:, 0:128], ones_t[:, 0:512],
                    start=(i == 0), stop=(i == 7),
                )

            # pair tiles: partitions 0:64 = even head dims, 64:128 = odd
            qp_t = [pp.tile([128, T], F16, tag=f"q{p}", name=f"q{p}")
                    for p in range(2)]
            kp_t = [pp.tile([128, T], F16, tag=f"k{p}", name=f"k{p}")
                    for p in range(2)]
            vp_t = [pp.tile([128, HPC * 65], F16, tag=f"v{i}", name=f"v{i}")
                    for i in range(NKT)]
            for i in range(NKT):
                nc.gpsimd.memset(
                    vp_t[i][:].rearrange("p (h c) -> p h c", c=65)[:, :, 64:65], 1.0)
            at_t = [pp.tile([128, T], F32R, tag=f"at{c}", name=f"at{c}")
                    for c in range(2)]

            def proj_group(j, m):
                # qkT[:, j-chunk] for one pair tile: m=0 -> Q pair 0,
                # m=1 -> K pair 0, m=2 -> Q pair 1, m=3 -> K pair 1
                ps = psp.tile([128, 512], F32, tag="misc", name="ps_proj", bufs=2)
                for k in range(8):
                    nc.tensor.matmul(
                        ps[:],
                        combv[:, k, m * 128:(m + 1) * 128],
                        combv[:, k, 768 + j * 512:768 + (j + 1) * 512],
                        start=(k == 0), stop=(k == 7),
                    )
                dst = (qp_t if m % 2 == 0 else kp_t)[m // 2]
                nc.vector.tensor_copy(dst[:, j * 512:(j + 1) * 512], ps[:])

            def v_tile(kt):
                ps = psp.tile([128, 256], F32, tag="misc", name="ps_v", bufs=2)
                for k in range(8):
                    nc.tensor.matmul(
                        ps[:],
                        combv[:, k, 768 + kt * 128:768 + (kt + 1) * 128],
                        combv[:, k, 512:768],
                        start=(k == 0), stop=(k == 7),
                    )
                vt = vp_t[kt]
                v_view = vt[:].rearrange("p (h c) -> p h c", c=65)
                ps_view = ps[:].rearrange("p (h c) -> p h c", c=64)
                nc.vector.tensor_copy(v_view[:, :, 0:64], ps_view[:])

            # per-chunk normalization state: ar tiles (SBUF copies of the
            # raw av, incl. the den row) and the packed den tile with head
            # h's denominators at partition 32*h.
            chunk_ars = {}
            chunk_den = {}
            chunk_rec = {}
            pending_norm = []

            def flush_norm():
                while pending_norm:
                    rec_f, rest_f = pending_norm.pop(0)
                    rec_f()
                    rest_f()

            def attn_pair(j, hp, fillers=()):
                """Attention for head pair hp at q-chunk j. Both heads' QK
                matmuls run K=64 row-tiled into one two-bank PSUM tile and
                execute concurrently in the PE array. fillers are PE-work
                thunks injected at evenly spaced kt slots."""
                nkt = 4 * j + 4
                qs = slice(j * 512, (j + 1) * 512)

                def score_pair(kt):
                    """QK + exp for both heads at one k-tile. Head A in
                    cols 0:w, head B in cols 512:512+w of a [128, 1024]
                    sp tile; one fused exp for full tiles. Diagonal tiles
                    (kt >= 4j) are column-restricted; the first 128
                    columns get the causal mask multiply. Returns per-head
                    AV operand lists [(src, out_col, width), ...]."""
                    d4 = kt - 4 * j
                    if d4 < 0:
                        c0, w = 0, 512
                    else:
                        c0, w = d4 * 128, 512 - d4 * 128
                    sp = psp.tile([128, 1024], F32, tag="sp", name="sp")
                    et = wp.tile([128, 1024], F16, tag="e", name="et")
                    for hh in range(2):
                        pb = 64 * hh
                        nc.tensor.matmul(
                            sp[:, 512 * hh:512 * hh + w],
                            kp_t[hp][pb:pb + 64, kt * 128:(kt + 1) * 128],
                            qp_t[hp][pb:pb + 64, j * 512 + c0:(j + 1) * 512],
                            start=True, stop=True,
                        )
                    if d4 < 0:
                        nc.scalar.activation(et[:], sp[:], AF.Exp, scale=0.125)
                        return [[(et[:, 0:512], 0, 512)],
                                [(et[:, 512:1024], 0, 512)]]
                    parts = []
                    for hh in range(2):
                        e0 = 512 * hh
                        nc.scalar.activation(et[:, e0:e0 + w],
                                             sp[:, e0:e0 + w],
                                             AF.Exp, scale=0.125)
                        emt = nwp.tile([128, 128], F16, tag="em", name="emt")
                        nc.vector.tensor_mul(emt[:], et[:, e0:e0 + 128],
                                             consts_t[:])
                        hp_parts = [(emt[:], c0, 128)]
                        if w > 128:
                            hp_parts.append((et[:, e0 + 128:e0 + w],
                                             c0 + 128, w - 128))
                        parts.append(hp_parts)
                    return parts

                fl = list(fillers)
                sched = {}
                for fi, f in enumerate(fl):
                    sched.setdefault((fi * nkt) // len(fl), []).append(f)

                avs = [psp.tile([65, 512], F32, tag="av", name=f"av{hh}")
                       for hh in range(2)]
                srcs = {0: score_pair(0)}
                for kt in range(nkt):
                    if kt + 1 < nkt:
                        srcs[kt + 1] = score_pair(kt + 1)
                    for f in sched.pop(kt, ()):
                        f()
                    pair_parts = srcs.pop(kt)
                    for hh in range(2):
                        h = 2 * hp + hh
                        for pi, (src, c0, w) in enumerate(pair_parts[hh]):
                            nc.tensor.matmul(
                                avs[hh][:, c0:c0 + w],
                                vp_t[kt][:, h * 65:(h + 1) * 65],
                                src,
                                start=(kt == 0),
                                stop=(kt == nkt - 1
                                      and pi == len(pair_parts[hh]) - 1),
                                skip_group_check=True,
                            )

                # den rows straight from the PSUM ones row to the packed den
                # tile, then raw av to SBUF (ar): frees the av PSUM banks and
                # decouples the rest of the normalization entirely.
                if hp == 0:
                    chunk_ars[j] = []
                    chunk_den[j] = nwp.tile([97, 512], F16, tag="den",
                                            name="den", bufs=2)
                    # unused rows feed the packed LN/EXP; 1.0 -> rec of 1.0
                    nc.gpsimd.memset(chunk_den[j][:], 1.0)
                for hh in range(2):
                    h = 2 * hp + hh
                    with nc.allow_low_precision(reason="softmax den"):
                        nc.vector.tensor_copy(
                            chunk_den[j][32 * h:32 * h + 1, :],
                            avs[hh][64:65, :])
                    ar = wp.tile([65, 512], F16, tag="ar", name="ar", bufs=8)
                    nc.vector.tensor_copy(ar[:], avs[hh][:])
                    chunk_ars[j].append(ar)

                if hp == 1:
                    den = chunk_den[j]
                    ars = chunk_ars[j]

                    def norm_rec(j=j, den=den):
                        # 1/den for all 4 heads in one packed ACT pass; both
                        # functions live in the natural_log_exp table set.
                        ln_t = nwp.tile([97, 512], F32, tag="ln", name="ln_t",
                                        bufs=2)
                        nc.scalar.activation(ln_t[:], den[:], AF.Ln)
                        rec = nwp.tile([97, 512], F16, tag="rec", name="rec",
                                       bufs=2)
                        with nc.allow_low_precision(reason="softmax recip"):
                            nc.scalar.activation(rec[:], ln_t[:], AF.Exp,
                                                 scale=-1.0)
                        chunk_rec[j] = rec

                    def norm_rest(j=j, ars=ars):
                        rec = chunk_rec[j]
                        for hp2 in range(2):
                            bc = psp.tile([128, 512], F32, tag="misc",
                                          name="bc", bufs=2)
                            for hh in range(2):
                                h = 2 * hp2 + hh
                                nc.tensor.matmul(
                                    bc[64 * hh:64 * hh + 64, :],
                                    ones_t[32 * h:32 * h + 1, 0:64],
                                    rec[32 * h:32 * h + 1, :],
                                    start=True, stop=True,
                                    tile_position=(32 * h, 64 * hh),
                                )
                            for hh in range(2):
                                h = 2 * hp2 + hh
                                arow = 64 * hh
                                with nc.allow_low_precision(
                                        reason="normalized attn"):
                                    nc.vector.tensor_mul(
                                        at_t[hp2][arow:arow + 64,
                                                  j * 512:(j + 1) * 512],
                                        ars[h][0:64, :],
                                        bc[arow:arow + 64, :],
                                    )

                    pending_norm.append((norm_rec, norm_rest))

            def wo_step(t, on_act=False):
                os = nwp.tile([128, D], F16, tag="os", name="os", bufs=3)
                for n in range(2):
                    wpb = psp.tile([128, 512], F32, tag="misc", name="wpb",
                                   bufs=2)
                    for c in range(2):
                        nc.tensor.matmul(
                            wpb[:],
                            at_t[c][:, t * 128:(t + 1) * 128],
                            wo_t[c][:, n * 512:(n + 1) * 512],
                            start=(c == 0), stop=(c == 1),
                        )
                    if on_act:
                        nc.scalar.copy(os[:, n * 512:(n + 1) * 512], wpb[:])
                    else:
                        nc.vector.tensor_copy(os[:, n * 512:(n + 1) * 512], wpb[:])
                eng = nc.scalar if on_act else nc.sync
                eng.dma_start(out[t * 128:(t + 1) * 128, :], os[:])

            # ---- schedule ----
            # Norm chains are chunk-deferred and split: chunk j's LN/EXP
            # (REC) flushes a slot into chunk j+1's first pair so it rides
            # the ACT queue after a few of that pair's exps (no ACT bubble
            # waiting on the den copies); the bc matmuls + mults (RES)
            # flush two slots later when rec is ready. wo steps follow the
            # RES of their chunk.
            def rec1():
                if pending_norm:
                    pending_norm[0][0]()

            def res1():
                if pending_norm:
                    pending_norm.pop(0)[1]()

            proj_group(0, 0)
            proj_group(0, 1)
            attn_pair(0, 0, (lambda: v_tile(0), lambda: v_tile(1),
                             lambda: v_tile(2), lambda: v_tile(3),
                             lambda: proj_group(0, 2), lambda: proj_group(0, 3)))
            attn_pair(0, 1, (lambda: proj_group(1, 0), lambda: proj_group(1, 1),
                             lambda: v_tile(4), lambda: v_tile(5)))
            attn_pair(1, 0, (rec1, lambda: proj_group(1, 2), res1,
                             lambda: v_tile(6), lambda: v_tile(7),
                             lambda: proj_group(1, 3),
                             lambda: wo_step(0), lambda: wo_step(1)))
            attn_pair(1, 1, (lambda: proj_group(2, 0), lambda: proj_group(2, 1),
                             lambda: wo_step(2), lambda: wo_step(3)))
            attn_pair(2, 0, (rec1, lambda: proj_group(2, 2), res1,
                             lambda: proj_group(2, 3),
                             lambda: v_tile(8), lambda: v_tile(9),
                             lambda: v_tile(10), lambda: v_tile(11),
                             lambda: wo_step(4)))
            attn_pair(2, 1, (lambda: proj_group(3, 0), lambda: proj_group(3, 1),
                             lambda: v_tile(12), lambda: v_tile(13),
                             lambda: wo_step(5), lambda: wo_step(6),
                             lambda: wo_step(7)))
            attn_pair(3, 0, (rec1, lambda: proj_group(3, 2), res1,
                             lambda: proj_group(3, 3),
                             lambda: v_tile(14), lambda: v_tile(15),
                             lambda: wo_step(8), lambda: wo_step(9),
                             lambda: wo_step(10)))
            attn_pair(3, 1, (lambda: wo_step(11),))
            flush_norm()
            for t in range(12, 16):
                wo_step(t, on_act=(t >= 14))
    return nc


def _make_masks():
    p = np.arange(128)[:, None]
    f = np.arange(128)[None, :]
    return (p <= f).astype(np.float16)


_NC_CACHE = {}


def make_in_maps(x, W_qkv, W_o):
    x = np.ascontiguousarray(np.asarray(x, dtype=np.float32))
    W_qkv = np.ascontiguousarray(np.asarray(W_qkv, dtype=np.float32))
    W_o = np.ascontiguousarray(np.asarray(W_o, dtype=np.float32))
    W_q, W_k, W_v = W_qkv[:, :D], W_qkv[:, D:2 * D], W_qkv[:, 2 * D:]
    masks = _make_masks()

    in_maps = []
    for c in range(N_CORES):
        b, g = c // 4, c % 4
        c0 = g * HL
        cxv = np.concatenate(
            [W_q[:, c0:c0 + 128], W_k[:, c0:c0 + 128],
             W_q[:, c0 + 128:c0 + 256], W_k[:, c0 + 128:c0 + 256],
             W_v[:, c0:c0 + 256], x[b].T], axis=1
        ).astype(np.float16)
        in_maps.append({
            "cx": np.ascontiguousarray(cxv),
            "wo": np.ascontiguousarray(W_o[g * HL:(g + 1) * HL, :]),
            "consts": masks,
        })
    return in_maps


def kernel(x, W_qkv, W_o):
    if "nc" not in _NC_CACHE:
        _NC_CACHE["nc"] = build_nc()
    nc = _NC_CACHE["nc"]

    in_maps = make_in_maps(x, W_qkv, W_o)
    res = run_bass_kernel_spmd(nc, in_maps, list(range(N_CORES)))
    out = np.zeros((B, T, D), dtype=np.float32)
    for c in range(N_CORES):
        out[c // 4] += res.results[c]["out"].astype(np.float32)
    return out


# revision 22
# speedup vs baseline: 1.2264x; 1.2264x over previous
"""Multi-head causal attention (B=2, T=2048, D=1024, H=16) on 8 Trainium2
NeuronCores.

Sharding: batch x head-group data/tensor parallel. Core c handles batch
c//4 and heads (c%4)*4 .. +4: W_qkv is split column-wise per head group,
W_o row-wise; each core computes attention for its local heads and a
partial output projection. The host sums the 4 partials per batch
(row-parallel W_o reduction) and stacks the two batches.

Per-core device kernel (fp16 data path, fp32 PSUM accumulate):
  - Inputs ride in as a handful of batched 3D-AP DMA waves (one
    dma_start per column range over all eight 128-row chunks of the
    [D, 2816] comb layout [Wq_p0|Wk_p0|Wq_p1|Wk_p1|Wv|xT]), spread
    across the SP/ACT/Pool DGE queues so descriptor issue (~600 ns
    each) never serializes the head.
  - Q/K live as head-PAIR tiles [128, T]: partitions 0:64 = even head
    dims, 64:128 = odd. The two QK matmuls of a pair run K=64
    row-tiled into ONE two-bank PSUM tile [128, 1024] (A cols 0:512,
    B cols 512:1024) so both banks free together and the matmuls
    execute concurrently in the PE array's row halves.
  - exp(s/8) on ACT: one fused [128, 1024] op per full k-tile (both
    heads), two ops for the column-restricted diagonal tiles; causal
    mask multiply on DVE for the diagonal 128-col block.
  - AV accumulates [65, 512] per head (V tiles carry a ones column so
    the softmax denominator falls out as row 64). Each av is copied to
    SBUF right away (ar tiles), releasing the PSUM bank, so the whole
    normalization chain runs off the critical path.
  - Normalization per chunk (4 heads packed at partitions 0/32/64/96):
    den rows gathered on GpSimd, 1/den = exp(-ln d) on ACT (one LN +
    one EXP per chunk), K=1 ones-matmul broadcast on PE into a shared
    [128, 512] bc tile (two heads per bank via out base partition),
    multiply ar * bc into the fp32r attnT on DVE.
  - W_o projection (fp32r) per 128-token step, PSUM -> SBUF fp16
    copies on DVE (tail: ACT), one batched [128, 1024] out-DMA per
    step.
  - Tail: per-proc drain waits + a single fan-out barrier + semaphore
    clear instead of the stock double butterfly barrier.
  All projection / V-tile / W_o / norm quanta ride as fillers at chosen
  slots inside the attention kt loops, sized so the PE queue stays
  dense while ACT works through the exp stream.
"""
import sys

for _p in ("/opt/trn_rl_repo", "/root/.axon_site/_ro/trn_rl_repo"):
    if _p not in sys.path:
        sys.path.insert(0, _p)

import numpy as np
import concourse.bass as bass
import concourse.mybir as mybir
import concourse.tile as tile
from concourse.vector_clock import ScopedClock
from concourse.bass_utils import run_bass_kernel_spmd

F32 = mybir.dt.float32
F32R = mybir.dt.float32r
F16 = mybir.dt.float16
AF = mybir.ActivationFunctionType

B, T, D = 2, 2048, 1024
N_CORES = 8
HPC = 4            # heads per core
HL = HPC * 64      # 256 local head dims
NKT = T // 128     # 16 k-tiles per head
NQC = T // 512     # 4 q-chunks
CW = 2816          # comb columns: Wq0|Wk0|Wq1|Wk1|Wv|xT


class FixedTileContext(tile.TileContext):
    """Works around this walrus build's 1-sync-wait-per-instruction limit.

    1. `_add_instruction`: peel extra waits off any instruction onto
       standalone single-wait nops emitted just before it on the same
       engine (the sequencer executes them in order).
    2. `_drain_and_barrier`: per-proc single-wait sync nops, then a
       single fan-out barrier (sync increments, every other engine takes
       one wait) instead of two full butterfly barriers, then the
       semaphore clear on gpsimd.
    """

    def _add_instruction(self, inst):
        si = inst.sync_info
        if si is not None:
            waits = list(si.on_wait)
            if len(waits) > 1:
                eng = getattr(inst, "engine", None)
                eng_obj = self.nc.engines.get(eng) if eng is not None else None
                if eng_obj is not None:
                    for w in waits[:-1]:
                        nop = eng_obj.nop()
                        nop.ins.sync_info = mybir.SyncInfo(on_wait=[w], on_update=[])
                    inst.sync_info = mybir.SyncInfo(
                        on_wait=[waits[-1]], on_update=list(si.on_update)
                    )
        super()._add_instruction(inst)

    def _drain_and_barrier(self, tick_clock, wait_clock):
        nc = self.nc
        vec = tick_clock.global_clock
        for proc in range(len(vec)):
            t = vec[proc]
            if t <= 0:
                continue
            partial = ScopedClock()
            partial.require_at_least(None, proc, t)
            w = nc.sync.nop()
            wait_clock.add_sem_waits(w.ins, partial)
        nc.sync.drain()
        # fan-out: sync (which waited on every proc above) releases the
        # other engines with one semaphore; each takes a single wait.
        bsem = nc.alloc_semaphore("tail_fanout")
        nc.sync.sem_inc(bsem)
        for eng in nc.engines.values():
            if eng is not nc.sync:
                eng.wait_ge(bsem, 1)
        assert self.sems is not None
        popped = nc._tile_sem_poison_stack.pop()
        assert popped is self._sem_poison
        sems = list(self.sems.allocated().values())
        nc.clear_and_free_semaphores(sems)
        nc.gpsimd.sem_clear(range(bsem.num, bsem.num + 1))


def build_nc():
    nc = bass.Bass()
    cx = nc.declare_dram_parameter("cx", [D, CW], F16, isOutput=False)
    wo = nc.declare_dram_parameter("wo", [HL, D], F32R, isOutput=False)
    consts = nc.declare_dram_parameter("consts", [128, 128], F16, isOutput=False)
    out = nc.declare_dram_parameter("out", [T, D], F16, isOutput=True)

    cxr = cx[:].rearrange("(k p) c -> p k c", p=128)      # [128, 8, 2816]

    with FixedTileContext(nc) as tc:
        with tc.tile_pool(name="persist", bufs=1) as pp, \
             tc.tile_pool(name="work", bufs=4) as wp, \
             tc.tile_pool(name="nwork", bufs=4) as nwp, \
             tc.tile_pool(name="psum", bufs=2, space="PSUM") as psp:
            comb = pp.tile([128, 8 * CW], F16, tag="comb")
            combv = comb[:].rearrange("p (k c) -> p k c", c=CW)
            consts_t = pp.tile([128, 128], F16, tag="consts")
            ones_t = pp.tile([128, 512], F16, tag="ones")
            wo_t = [pp.tile([128, D], F32R, tag=f"wo{c}", name=f"wo{c}")
                    for c in range(2)]

            # ---- input DMA waves ----
            # All input waves ride ONE engine (sync) in strict priority
            # order: concurrent waves from several engines fair-share the
            # DMA queues, which dilutes the critical first-chunk columns;
            # a single prioritized chain gives each wave the full fabric
            # bandwidth (~330 GB/s) in turn. Transfers can't start before
            # the ~7.5 us sequencer preamble regardless of issue engine.
            nc.scalar.dma_start(consts_t[:], consts[:])
            nc.gpsimd.memset(ones_t[:], 1.0)     # warmup dep
            nc.sync.dma_start(combv[:, :, 0:256], cxr[:, :, 0:256])
            nc.sync.dma_start(combv[:, :, 512:768], cxr[:, :, 512:768])
            nc.sync.dma_start(combv[:, :, 768:1280], cxr[:, :, 768:1280])
            nc.sync.dma_start(combv[:, :, 256:512], cxr[:, :, 256:512])
            nc.sync.dma_start(combv[:, :, 1280:1792], cxr[:, :, 1280:1792])
            for c in range(2):
                nc.sync.dma_start(wo_t[c][:], wo[c * 128:(c + 1) * 128, :])
            nc.sync.dma_start(combv[:, :, 1792:2304], cxr[:, :, 1792:2304])
            nc.sync.dma_start(combv[:, :, 2304:2816], cxr[:, :, 2304:2816])

            # HAM warm-up: dummy N=512 accumulation chain on the ones tile
            # keeps the PE busy while the first input waves land so the
            # clock gate opens (cold K=4/8 halves the PE clock).
            wu = psp.tile([128, 512], F32, tag="misc", name="wu", bufs=2)
            for i in range(12):
                nc.tensor.matmul(
                    wu[:], ones_t[:, 0:128], ones_t[:, 0:512],
                    start=(i == 0), stop=(i == 11),
                )

            # pair tiles: partitions 0:64 = even head dims, 64:128 = odd
            qp_t = [pp.tile([128, T], F16, tag=f"q{p}", name=f"q{p}")
                    for p in range(2)]
            kp_t = [pp.tile([128, T], F16, tag=f"k{p}", name=f"k{p}")
                    for p in range(2)]
            vp_t = [pp.tile([128, HPC * 65], F16, tag=f"v{i}", name=f"v{i}")
                    for i in range(NKT)]
            for i in range(NKT):
                nc.gpsimd.memset(
                    vp_t[i][:].rearrange("p (h c) -> p h c", c=65)[:, :, 64:65], 1.0)
            at_t = [pp.tile([128, T], F32R, tag=f"at{c}", name=f"at{c}")
                    for c in range(2)]

            def proj_group(j, m):
                # qkT[:, j-chunk] for one pair tile: m=0 -> Q pair 0,
                # m=1 -> K pair 0, m=2 -> Q pair 1, m=3 -> K pair 1
                ps = psp.tile([128, 512], F32, tag="misc", name="ps_proj", bufs=2)
                for k in range(8):
                    nc.tensor.matmul(
                        ps[:],
                        combv[:, k, m * 128:(m + 1) * 128],
                        combv[:, k, 768 + j * 512:768 + (j + 1) * 512],
                        start=(k == 0), stop=(k == 7),
                    )
                dst = (qp_t if m % 2 == 0 else kp_t)[m // 2]
                nc.vector.tensor_copy(dst[:, j * 512:(j + 1) * 512], ps[:])

            def v_tile(kt):
                ps = psp.tile([128, 256], F32, tag="misc", name="ps_v", bufs=2)
                for k in range(8):
                    nc.tensor.matmul(
                        ps[:],
                        combv[:, k, 768 + kt * 128:768 + (kt + 1) * 128],
                        combv[:, k, 512:768],
                        start=(k == 0), stop=(k == 7),
                    )
                vt = vp_t[kt]
                v_view = vt[:].rearrange("p (h c) -> p h c", c=65)
                ps_view = ps[:].rearrange("p (h c) -> p h c", c=64)
                nc.vector.tensor_copy(v_view[:, :, 0:64], ps_view[:])

            # per-chunk normalization state: ar tiles (SBUF copies of the
            # raw av, incl. the den row) and the packed den tile with head
            # h's denominators at partition 32*h.
            chunk_ars = {}
            chunk_den = {}
            chunk_rec = {}
            pending_norm = []

            def flush_norm():
                while pending_norm:
                    rec_f, rest_f = pending_norm.pop(0)
                    rec_f()
                    rest_f()

            def attn_pair(j, hp, fillers=()):
                """Attention for head pair hp at q-chunk j. Both heads' QK
                matmuls run K=64 row-tiled into one two-bank PSUM tile and
                execute concurrently in the PE array. fillers are PE-work
                thunks injected at evenly spaced kt slots."""
                nkt = 4 * j + 4
                qs = slice(j * 512, (j + 1) * 512)

                def score_pair(kt):
                    """QK + exp for both heads at one k-tile. Head A in
                    cols 0:w, head B in cols 512:512+w of a [128, 1024]
                    sp tile; one fused exp for full tiles. Diagonal tiles
                    (kt >= 4j) are column-restricted; the first 128
                    columns get the causal mask multiply. Returns per-head
                    AV operand lists [(src, out_col, width), ...]."""
                    d4 = kt - 4 * j
                    if d4 < 0:
                        c0, w = 0, 512
                    else:
                        c0, w = d4 * 128, 512 - d4 * 128
                    sp = psp.tile([128, 1024], F32, tag="sp", name="sp")
                    et = wp.tile([128, 1024], F16, tag="e", name="et")
                    for hh in range(2):
                        pb = 64 * hh
                        nc.tensor.matmul(
                            sp[:, 512 * hh:512 * hh + w],
                            kp_t[hp][pb:pb + 64, kt * 128:(kt + 1) * 128],
                            qp_t[hp][pb:pb + 64, j * 512 + c0:(j + 1) * 512],
                            start=True, stop=True,
                        )
                    if d4 < 0:
                        nc.scalar.activation(et[:], sp[:], AF.Exp, scale=0.125)
                        return [[(et[:, 0:512], 0, 512)],
                                [(et[:, 512:1024], 0, 512)]]
                    parts = []
                    for hh in range(2):
                        e0 = 512 * hh
                        nc.scalar.activation(et[:, e0:e0 + w],
                                             sp[:, e0:e0 + w],
                                             AF.Exp, scale=0.125)
                        emt = nwp.tile([128, 128], F16, tag="em", name="emt")
                        nc.vector.tensor_mul(emt[:], et[:, e0:e0 + 128],
                                             consts_t[:])
                        hp_parts = [(emt[:], c0, 128)]
                        if w > 128:
                            hp_parts.append((et[:, e0 + 128:e0 + w],
                                             c0 + 128, w - 128))
                        parts.append(hp_parts)
                    return parts

                fl = list(fillers)
                sched = {}
                for fi, f in enumerate(fl):
                    sched.setdefault((fi * nkt) // len(fl), []).append(f)

                avs = [psp.tile([65, 512], F32, tag="av", name=f"av{hh}")
                       for hh in range(2)]
                srcs = {0: score_pair(0)}
                for kt in range(nkt):
                    if kt + 1 < nkt:
                        srcs[kt + 1] = score_pair(kt + 1)
                    for f in sched.pop(kt, ()):
                        f()
                    pair_parts = srcs.pop(kt)
                    for hh in range(2):
                        h = 2 * hp + hh
                        for pi, (src, c0, w) in enumerate(pair_parts[hh]):
                            nc.tensor.matmul(
                                avs[hh][:, c0:c0 + w],
                                vp_t[kt][:, h * 65:(h + 1) * 65],
                                src,
                                start=(kt == 0),
                                stop=(kt == nkt - 1
                                      and pi == len(pair_parts[hh]) - 1),
                                skip_group_check=True,
                            )

                # den rows straight from the PSUM ones row to the packed den
                # tile, then raw av to SBUF (ar): frees the av PSUM banks and
                # decouples the rest of the normalization entirely.
                if hp == 0:
                    chunk_ars[j] = []
                    chunk_den[j] = nwp.tile([97, 512], F16, tag="den",
                                            name="den", bufs=2)
                    # unused rows feed the packed LN/EXP; 1.0 -> rec of 1.0
                    nc.gpsimd.memset(chunk_den[j][:], 1.0)
                for hh in range(2):
                    h = 2 * hp + hh
                    with nc.allow_low_precision(reason="softmax den"):
                        nc.vector.tensor_copy(
                            chunk_den[j][32 * h:32 * h + 1, :],
                            avs[hh][64:65, :])
                    ar = wp.tile([65, 512], F16, tag="ar", name="ar", bufs=8)
                    nc.vector.tensor_copy(ar[:], avs[hh][:])
                    chunk_ars[j].append(ar)

                if hp == 1:
                    den = chunk_den[j]
                    ars = chunk_ars[j]

                    def norm_rec(j=j, den=den):
                        # 1/den for all 4 heads in one packed ACT pass; both
                        # functions live in the natural_log_exp table set.
                        ln_t = nwp.tile([97, 512], F32, tag="ln", name="ln_t",
                                        bufs=2)
                        nc.scalar.activation(ln_t[:], den[:], AF.Ln)
                        rec = nwp.tile([97, 512], F16, tag="rec", name="rec",
                                       bufs=2)
                        with nc.allow_low_precision(reason="softmax recip"):
                            nc.scalar.activation(rec[:], ln_t[:], AF.Exp,
                                                 scale=-1.0)
                        chunk_rec[j] = rec

                    def norm_rest(j=j, ars=ars):
                        rec = chunk_rec[j]
                        for hp2 in range(2):
                            bc = psp.tile([128, 512], F32, tag="misc",
                                          name="bc", bufs=2)
                            for hh in range(2):
                                h = 2 * hp2 + hh
                                nc.tensor.matmul(
                                    bc[64 * hh:64 * hh + 64, :],
                                    ones_t[32 * h:32 * h + 1, 0:64],
                                    rec[32 * h:32 * h + 1, :],
                                    start=True, stop=True,
                                    tile_position=(32 * h, 64 * hh),
                                )
                            for hh in range(2):
                                h = 2 * hp2 + hh
                                arow = 64 * hh
                                with nc.allow_low_precision(
                                        reason="normalized attn"):
                                    nc.vector.tensor_mul(
                                        at_t[hp2][arow:arow + 64,
                                                  j * 512:(j + 1) * 512],
                                        ars[h][0:64, :],
                                        bc[arow:arow + 64, :],
                                    )

                    pending_norm.append((norm_rec, norm_rest))

            def wo_step(t, on_act=False):
                os = nwp.tile([128, D], F16, tag="os", name="os", bufs=3)
                for n in range(2):
                    wpb = psp.tile([128, 512], F32, tag="misc", name="wpb",
                                   bufs=2)
                    for c in range(2):
                        nc.tensor.matmul(
                            wpb[:],
                            at_t[c][:, t * 128:(t + 1) * 128],
                            wo_t[c][:, n * 512:(n + 1) * 512],
                            start=(c == 0), stop=(c == 1),
                        )
                    if on_act:
                        nc.scalar.copy(os[:, n * 512:(n + 1) * 512], wpb[:])
                    else:
                        nc.vector.tensor_copy(os[:, n * 512:(n + 1) * 512], wpb[:])
                eng = nc.scalar if on_act else nc.sync
                eng.dma_start(out[t * 128:(t + 1) * 128, :], os[:])

            # ---- schedule ----
            # Norm chains are chunk-deferred and split: chunk j's LN/EXP
            # (REC) flushes a slot into chunk j+1's first pair so it rides
            # the ACT queue after a few of that pair's exps (no ACT bubble
            # waiting on the den copies); the bc matmuls + mults (RES)
            # flush two slots later when rec is ready. wo steps follow the
            # RES of their chunk.
            def rec1():
                if pending_norm:
                    pending_norm[0][0]()

            def res1():
                if pending_norm:
                    pending_norm.pop(0)[1]()

            proj_group(0, 0)
            proj_group(0, 1)
            attn_pair(0, 0, (lambda: v_tile(0), lambda: v_tile(1),
                             lambda: v_tile(2), lambda: v_tile(3),
                             lambda: proj_group(0, 2), lambda: proj_group(0, 3)))
            attn_pair(0, 1, (lambda: proj_group(1, 0), lambda: proj_group(1, 1),
                             lambda: v_tile(4), lambda: v_tile(5)))
            attn_pair(1, 0, (rec1, lambda: proj_group(1, 2), res1,
                             lambda: v_tile(6), lambda: v_tile(7),
                             lambda: proj_group(1, 3),
                             lambda: wo_step(0), lambda: wo_step(1)))
            attn_pair(1, 1, (lambda: proj_group(2, 0), lambda: proj_group(2, 1),
                             lambda: wo_step(2), lambda: wo_step(3)))
            attn_pair(2, 0, (rec1, lambda: proj_group(2, 2), res1,
                             lambda: proj_group(2, 3),
                             lambda: v_tile(8), lambda: v_tile(9),
                             lambda: v_tile(10), lambda: v_tile(11),
                             lambda: wo_step(4)))
            attn_pair(2, 1, (lambda: proj_group(3, 0), lambda: proj_group(3, 1),
                             lambda: v_tile(12), lambda: v_tile(13),
                             lambda: wo_step(5), lambda: wo_step(6),
                             lambda: wo_step(7)))
            attn_pair(3, 0, (rec1, lambda: proj_group(3, 2), res1,
                             lambda: proj_group(3, 3),
                             lambda: v_tile(14), lambda: v_tile(15),
                             lambda: wo_step(8), lambda: wo_step(9),
                             lambda: wo_step(10)))
            attn_pair(3, 1, (lambda: wo_step(11),))
            flush_norm()
            for t in range(12, 16):
                wo_step(t, on_act=(t >= 14))
    return nc


def _make_masks():
    p = np.arange(128)[:, None]
    f = np.arange(128)[None, :]
    return (p <= f).astype(np.float16)


_NC_CACHE = {}


def make_in_maps(x, W_qkv, W_o):
    x = np.ascontiguousarray(np.asarray(x, dtype=np.float32))
    W_qkv = np.ascontiguousarray(np.asarray(W_qkv, dtype=np.float32))
    W_o = np.ascontiguousarray(np.asarray(W_o, dtype=np.float32))
    W_q, W_k, W_v = W_qkv[:, :D], W_qkv[:, D:2 * D], W_qkv[:, 2 * D:]
    masks = _make_masks()

    in_maps = []
    for c in range(N_CORES):
        b, g = c // 4, c % 4
        c0 = g * HL
        cxv = np.concatenate(
            [W_q[:, c0:c0 + 128], W_k[:, c0:c0 + 128],
             W_q[:, c0 + 128:c0 + 256], W_k[:, c0 + 128:c0 + 256],
             W_v[:, c0:c0 + 256], x[b].T], axis=1
        ).astype(np.float16)
        in_maps.append({
            "cx": np.ascontiguousarray(cxv),
            "wo": np.ascontiguousarray(W_o[g * HL:(g + 1) * HL, :]),
            "consts": masks,
        })
    return in_maps


def kernel(x, W_qkv, W_o):
    if "nc" not in _NC_CACHE:
        _NC_CACHE["nc"] = build_nc()
    nc = _NC_CACHE["nc"]

    in_maps = make_in_maps(x, W_qkv, W_o)
    res = run_bass_kernel_spmd(nc, in_maps, list(range(N_CORES)))
    out = np.zeros((B, T, D), dtype=np.float32)
    for c in range(N_CORES):
        out[c // 4] += res.results[c]["out"].astype(np.float32)
    return out
